# revision 1
# baseline (speedup 1.0000x reference)
"""GATv2 2-layer GNN on 8 Trainium2 NeuronCores (Bass/Tile).

Strategy (dst-sharded edge parallelism):
- Nodes are sharded by destination across 8 cores (6250 nodes/core); each core
  owns all edges into its nodes, so segment softmax and aggregation are
  core-local.
- Per core, nodes are packed into 49 chunks of 128 slots (LPT balance on edge
  counts). The global slot id g(v) = core*6272 + pos permutes all feature
  tables; the host un-permutes the final output.
- Per layer: each core computes its projection shard on the PE, an AllGather
  replicates the [50176, 256] table, then the chunk loop: dma_gather of source
  rows (int16 pair-index trick over a [25088, 512] view; even/odd slot parity
  selects the base offset), one-hot expansion matmuls build xi+xj in PSUM,
  Prelu(0.2) on ACT, att-dot via bf16 mul+reduce on DVE, Exp on ACT, message
  scaling via a broadcast tensor_tensor, and a one-hot scatter matmul
  accumulates messages and softmax denominators in PSUM. The epilogue divides
  by the denominator, applies ReLU, and writes transposed copies for the next
  linear layer.
"""

import numpy as np

N = 50000
E = 800000
IN = 128
HC = 256          # H * HID
H = 4
C64 = 64
OUT = 64
SLOPE = 0.2
NCORES = 8
NPC = N // NCORES          # 6250 nodes per core
CHUNKS = 49                # ceil(6250/128)
P = 128
SHARD = CHUNKS * P         # 6272 slots per core
GSLOTS = NCORES * SHARD    # 50176
PAD_DST = 255.0            # dst slot sentinel for pad edges


# ---------------------------------------------------------------- host prep

def _pack_core(dst_local, n_nodes=NPC, chunks=CHUNKS):
    """LPT-pack nodes into `chunks` bins of <=128 nodes, balancing edges."""
    deg = np.bincount(dst_local, minlength=n_nodes)
    order = np.argsort(-deg, kind="stable")
    bin_load = np.zeros(chunks, np.int64)
    bin_cnt = np.zeros(chunks, np.int32)
    bin_members = [[] for _ in range(chunks)]
    for v in order:
        cand = np.where(bin_cnt < P)[0]
        b = cand[np.argmin(bin_load[cand])]
        bin_members[b].append(v)
        bin_load[b] += deg[v]
        bin_cnt[b] += 1
    perm = np.full(chunks * P, -1, np.int64)
    for b in range(chunks):
        for k, v in enumerate(bin_members[b]):
            perm[b * P + k] = v
    return perm


def _wrap_idx(flat):
    """[n] -> [128, n//16] int16 wrapped (i at [i%16, i//16]) + 8x replicated."""
    n = flat.shape[0]
    w = flat.reshape(n // 16, 16).T.astype(np.int16)
    return np.tile(w, (8, 1)).copy()


def prepare(inputs):
    import ml_dtypes
    x = np.asarray(inputs["x"], np.float32)
    ei = np.asarray(inputs["edge_index"]).astype(np.int64)
    src, dst = ei[0], ei[1]
    owner = dst // NPC
    dst_local = dst - owner * NPC

    perms = []
    pos_of = np.empty(N, np.int64)
    for c in range(NCORES):
        m = owner == c
        perm = _pack_core(dst_local[m])
        perms.append(perm)
        valid = perm >= 0
        pos_of[perm[valid] + c * NPC] = np.nonzero(valid)[0] + c * SHARD
    gsrc = pos_of[src]
    gdst = pos_of[dst]

    chunk_of_edge = (gdst % SHARD) // P
    slot_of_edge = gdst % P
    par = (gsrc & 1).astype(np.int64)

    ev_lists = [[[] for _ in range(CHUNKS)] for _ in range(NCORES)]
    od_lists = [[[] for _ in range(CHUNKS)] for _ in range(NCORES)]
    for e in range(E):
        tgt = ev_lists if par[e] == 0 else od_lists
        tgt[owner[e]][chunk_of_edge[e]].append(e)

    t_ev = max(max((len(l) + P - 1) // P for l in ev_lists[c]) for c in range(NCORES))
    t_od = max(max((len(l) + P - 1) // P for l in od_lists[c]) for c in range(NCORES))
    t_ev = max(t_ev, 1)
    t_od = max(t_od, 1)

    ev_idx = np.zeros((NCORES, CHUNKS, 128, t_ev * 8), np.int16)
    od_idx = np.zeros((NCORES, CHUNKS, 128, t_od * 8), np.int16)
    dst_meta = np.full((NCORES, CHUNKS, 128, t_ev + t_od), PAD_DST, np.float32)

    for c in range(NCORES):
        for j in range(CHUNKS):
            for edges, tcnt, arr, tb in (
                (ev_lists[c][j], t_ev, ev_idx, 0),
                (od_lists[c][j], t_od, od_idx, t_ev),
            ):
                flat = np.zeros(tcnt * P, np.int64)
                flat[: len(edges)] = gsrc[edges] >> 1
                arr[c, j] = _wrap_idx(flat)
                for i, e in enumerate(edges):
                    dst_meta[c, j, i % P, tb + i // P] = slot_of_edge[e]

    xs = []
    for c in range(NCORES):
        xp = np.zeros((SHARD, IN), np.float32)
        valid = perms[c] >= 0
        xp[valid] = x[perms[c][valid] + c * NPC]
        xs.append(np.ascontiguousarray(xp.T))

    W1T = np.ascontiguousarray(np.asarray(inputs["W1"], np.float32).T)
    W2T = np.ascontiguousarray(np.asarray(inputs["W2"], np.float32).T)
    W3T = np.ascontiguousarray(np.asarray(inputs["W3"], np.float32).T)
    W4T = np.ascontiguousarray(np.asarray(inputs["W4"], np.float32).T)
    att1b = np.tile(np.asarray(inputs["att1"], np.float32).reshape(1, 1, HC),
                    (P, 1, 1)).astype(ml_dtypes.bfloat16)
    att2b = np.tile(np.asarray(inputs["att2"], np.float32).reshape(1, 1, HC),
                    (P, 1, 1)).astype(ml_dtypes.bfloat16)

    in_maps = []
    for c in range(NCORES):
        in_maps.append({
            "x_perm_T": xs[c],
            "W1T": W1T, "W2T": W2T, "W3T": W3T, "W4T": W4T,
            "b1_row": np.asarray(inputs["b1"], np.float32).reshape(1, HC),
            "b2_row": np.asarray(inputs["b2"], np.float32).reshape(1, HC),
            "b3_row": np.asarray(inputs["b3"], np.float32).reshape(1, OUT),
            "b4_row": np.asarray(inputs["b4"], np.float32).reshape(1, OUT),
            "att1_b": att1b, "att2_b": att2b,
            "ev_idx": ev_idx[c], "od_idx": od_idx[c], "dst_meta": dst_meta[c],
        })
    return in_maps, perms, t_ev, t_od


# ------------------------------------------------------------- device build

def build(t_ev, t_od, upto='full', variant='full'):
    import concourse.bacc as bacc
    import concourse.mybir as mybir
    import concourse.tile as tile
    from concourse.masks import make_identity

    dt = mybir.dt
    AF = mybir.ActivationFunctionType
    ALU = mybir.AluOpType
    AX = mybir.AxisListType

    T = t_ev + t_od
    nc = bacc.Bacc("TRN2", target_bir_lowering=False, debug=False,
                   num_devices=NCORES)

    x_perm_T = nc.dram_tensor("x_perm_T", [P, SHARD], dt.float32, kind="ExternalInput")
    W1T = nc.dram_tensor("W1T", [IN, HC], dt.float32, kind="ExternalInput")
    W2T = nc.dram_tensor("W2T", [HC, HC], dt.float32, kind="ExternalInput")
    W3T = nc.dram_tensor("W3T", [HC, OUT], dt.float32, kind="ExternalInput")
    W4T = nc.dram_tensor("W4T", [OUT, OUT], dt.float32, kind="ExternalInput")
    b1_row = nc.dram_tensor("b1_row", [1, HC], dt.float32, kind="ExternalInput")
    b2_row = nc.dram_tensor("b2_row", [1, HC], dt.float32, kind="ExternalInput")
    b3_row = nc.dram_tensor("b3_row", [1, OUT], dt.float32, kind="ExternalInput")
    b4_row = nc.dram_tensor("b4_row", [1, OUT], dt.float32, kind="ExternalInput")
    att1_b = nc.dram_tensor("att1_b", [P, 1, HC], dt.bfloat16, kind="ExternalInput")
    att2_b = nc.dram_tensor("att2_b", [P, 1, HC], dt.bfloat16, kind="ExternalInput")
    ev_idx = nc.dram_tensor("ev_idx", [CHUNKS, 128, t_ev * 8], dt.int16, kind="ExternalInput")
    od_idx = nc.dram_tensor("od_idx", [CHUNKS, 128, t_od * 8], dt.int16, kind="ExternalInput")
    dst_meta = nc.dram_tensor("dst_meta", [CHUNKS, 128, T], dt.float32, kind="ExternalInput")
    y_shard = nc.dram_tensor("y_shard", [SHARD, OUT], dt.float32, kind="ExternalOutput")
    dbg_h = None
    if upto in ("A", "L1", "B", "L2"):
        dbg_h = nc.dram_tensor("dbg_h", [2 * P, HC], dt.float32, kind="ExternalOutput")

    h_shard = nc.dram_tensor("h_shard", [SHARD, HC], dt.float32)
    h_full = nc.dram_tensor("h_full", [GSLOTS, HC], dt.float32, addr_space="Shared")
    relu_T = nc.dram_tensor("relu_T", [HC, SHARD], dt.float32)

    rg = [list(range(NCORES))]

    with tile.TileContext(nc, num_cores=NCORES) as tc:
        with tc.tile_pool(name="const", bufs=1) as constp:
            ident = constp.tile([P, P], dt.float32)
            make_identity(nc, ident[:])
            iota_col = constp.tile([P, 1], dt.float32)
            nc.gpsimd.iota(iota_col[:], pattern=[[0, 1]], base=0,
                           channel_multiplier=1,
                           allow_small_or_imprecise_dtypes=True)
            iota_row = constp.tile([P, P], dt.float32)
            nc.gpsimd.iota(iota_row[:], pattern=[[1, P]], base=0,
                           channel_multiplier=0,
                           allow_small_or_imprecise_dtypes=True)
            ones_row = constp.tile([1, P], dt.float32)
            nc.gpsimd.memset(ones_row[:], 1.0)

            att_t = {1: constp.tile([P, 1, HC], dt.bfloat16, name="att_t1"),
                     2: constp.tile([P, 1, HC], dt.bfloat16, name="att_t2")}
            nc.sync.dma_start(out=att_t[1][:], in_=att1_b[:])
            nc.sync.dma_start(out=att_t[2][:], in_=att2_b[:])
            bias_t = {}
            for name, t_, w in (("b1", b1_row, HC), ("b2", b2_row, HC),
                                ("b3", b3_row, OUT), ("b4", b4_row, OUT)):
                bt = constp.tile([1, w], dt.float32, name=f"bias_{name}")
                nc.sync.dma_start(out=bt[:], in_=t_[:])
                bias_t[name] = bt
            wtile = {}
            for name, t_, kk, w in (("w1", W1T, IN, HC),
                                    ("w2lo", W2T[0:P, :], P, HC),
                                    ("w2hi", W2T[P:2 * P, :], P, HC),
                                    ("w3lo", W3T[0:P, :], P, OUT),
                                    ("w3hi", W3T[P:2 * P, :], P, OUT),
                                    ("w4", W4T, OUT, OUT)):
                wt = constp.tile([kk, w], dt.float32, name=f"w_{name}")
                nc.sync.dma_start(out=wt[:], in_=t_ if name not in ("w1", "w4") else t_[:])
                wtile[name] = wt

            def linear_phase(src_T_tiles, wts, bias, out_dram, width):
                with (
                    tc.tile_pool(name="linsb", bufs=3) as lsb,
                    tc.tile_pool(name="linps", bufs=2, space="PSUM") as lps,
                ):
                    for j in range(CHUNKS):
                        ps = lps.tile([P, width], dt.float32, tag="linps")
                        for ki, (sT, wT) in enumerate(zip(src_T_tiles, wts)):
                            nc.tensor.matmul(
                                out=ps[:], lhsT=sT[:, j * P:(j + 1) * P],
                                rhs=wT[:], start=(ki == 0), stop=False)
                        nc.tensor.matmul(out=ps[:], lhsT=ones_row[:],
                                         rhs=bias[:], start=False, stop=True)
                        ot = lsb.tile([P, width], dt.float32, tag="linout")
                        nc.scalar.activation(ot[:], ps[:], AF.Copy)
                        nc.sync.dma_start(out=out_dram[j * P:(j + 1) * P, :],
                                          in_=ot[:])

            def edge_layer(att_tile):
                pairs = h_full[:].rearrange("(a b) d -> a (b d)", b=2)
                with (
                    tc.tile_pool(name="chio", bufs=2) as chio,
                    tc.tile_pool(name="work", bufs=2) as work,
                    tc.tile_pool(name="eps", bufs=1, space="PSUM") as eps,
                ):
                    for j in range(CHUNKS):
                        hck = chio.tile([P, HC], dt.float32, tag="hchunk")
                        # own shard rows == own slice of h_full (pre-gather copy)
                        nc.sync.dma_start(out=hck[:],
                                          in_=h_shard[j * P:(j + 1) * P, :])
                        evi = chio.tile([128, t_ev * 8], dt.int16, tag="evi")
                        nc.sync.dma_start(out=evi[:], in_=ev_idx[j])
                        odi = chio.tile([128, t_od * 8], dt.int16, tag="odi")
                        nc.sync.dma_start(out=odi[:], in_=od_idx[j])
                        dmt = chio.tile([128, T], dt.float32, tag="dmt")
                        nc.sync.dma_start(out=dmt[:], in_=dst_meta[j])
                        xj_ev = chio.tile([P, t_ev, HC], dt.float32, tag="xjev")
                        xj_od = chio.tile([P, t_od, HC], dt.float32, tag="xjod")
                        if variant == "nogather":
                            nc.vector.tensor_copy(xj_ev[:, 0:1, :], hck[:].rearrange("p (o d) -> p o d", o=1))
                            nc.vector.tensor_copy(xj_od[:, 0:1, :], hck[:].rearrange("p (o d) -> p o d", o=1))
                        else:
                            nc.gpsimd.dma_gather(
                                out_ap=xj_ev[:], in_ap=pairs[:, 0:HC], idxs_ap=evi[:],
                                num_idxs=t_ev * P, num_idxs_reg=t_ev * P,
                                elem_size=HC, elem_step=2 * HC,
                                single_packet=False)
                            nc.gpsimd.dma_gather(
                                out_ap=xj_od[:], in_ap=pairs[:, HC:2 * HC], idxs_ap=odi[:],
                                num_idxs=t_od * P, num_idxs_reg=t_od * P,
                                elem_size=HC, elem_step=2 * HC,
                                single_packet=False)

                        msgden = eps.tile([P, HC + 4], dt.float32, tag="msgden",
                                          bufs=1)

                        groups = []
                        for base, tcnt, pool in ((0, t_ev, xj_ev),
                                                 (t_ev, t_od, xj_od)):
                            t0 = 0
                            while t0 < tcnt:
                                gb = min(3, tcnt - t0)
                                groups.append((base, t0, gb, pool))
                                t0 += gb

                        if variant == "nocompute":
                            groups = []
                        first = True
                        for (base, t0, gb, pool) in groups:
                            dstb = eps.tile([P, gb * P], dt.float32, tag="dstb",
                                            bufs=2)
                            for i in range(gb):
                                nc.tensor.transpose(
                                    out=dstb[:, i * P:(i + 1) * P],
                                    in_=dmt[:, base + t0 + i:base + t0 + i + 1]
                                        .to_broadcast([P, P]),
                                    identity=ident[:])
                            s_T = work.tile([P, gb * P], dt.float32, tag="s_T")
                            nc.vector.tensor_scalar(
                                out=s_T[:], in0=dstb[:], scalar1=iota_col[:, :1],
                                scalar2=None, op0=ALU.is_equal)
                            zp = eps.tile([P, gb * HC], dt.float32, tag="zp",
                                          bufs=2)
                            for i in range(gb):
                                nc.tensor.matmul(
                                    out=zp[:, i * HC:(i + 1) * HC],
                                    lhsT=s_T[:, i * P:(i + 1) * P], rhs=hck[:],
                                    start=True, stop=False)
                                nc.tensor.matmul(
                                    out=zp[:, i * HC:(i + 1) * HC],
                                    lhsT=ident[:], rhs=pool[:, t0 + i, :],
                                    start=False, stop=True)
                            s_b = work.tile([P, gb * HC], dt.bfloat16, tag="s_b")
                            nc.scalar.activation(s_b[:], zp[:], AF.Prelu,
                                                 alpha=SLOPE)
                            t_b = work.tile([P, gb * HC], dt.bfloat16, tag="t_b")
                            nc.vector.tensor_tensor(
                                out=t_b[:].rearrange("p (g d) -> p g d", g=gb),
                                in0=s_b[:].rearrange("p (g d) -> p g d", g=gb),
                                in1=att_tile[:].to_broadcast([P, gb, HC]),
                                op=ALU.mult)
                            alph = work.tile([P, gb * 4], dt.float32, tag="alph")
                            nc.vector.tensor_reduce(
                                out=alph[:].rearrange("p (g h) -> p g h", g=gb),
                                in_=t_b[:].rearrange("p (g h c) -> p g h c",
                                                     g=gb, h=H),
                                axis=AX.X, op=ALU.add)
                            msge = work.tile([P, gb, HC + 4], dt.float32,
                                             tag="msge")
                            nc.scalar.activation(
                                msge[:, :, HC:HC + 4],
                                alph[:].rearrange("p (g h) -> p g h", g=gb),
                                AF.Exp)
                            nc.vector.tensor_tensor(
                                out=msge[:, :, 0:HC].rearrange(
                                    "p g (h c) -> p g h c", h=H),
                                in0=pool[:, t0:t0 + gb, :].rearrange(
                                    "p g (h c) -> p g h c", h=H),
                                in1=msge[:, :, HC:HC + 4].to_broadcast(
                                    [P, gb, H, C64]),
                                op=ALU.mult)
                            for i in range(gb):
                                s_en = work.tile([P, P], dt.float32, tag="s_en")
                                nc.gpsimd.tensor_scalar(
                                    out=s_en[:], in0=iota_row[:],
                                    scalar1=dmt[:, base + t0 + i:base + t0 + i + 1],
                                    scalar2=None, op0=ALU.is_equal)
                                nc.tensor.matmul(
                                    out=msgden[:], lhsT=s_en[:],
                                    rhs=msge[:, i, :],
                                    start=first, stop=(base + t0 + i == T - 1))
                                first = False

                        if variant == "nocompute":
                            nc.tensor.matmul(out=msgden[:, 0:HC], lhsT=ident[:],
                                             rhs=xj_ev[:, 0, :],
                                             start=True, stop=True)
                            nc.tensor.matmul(out=msgden[:, HC:HC + 4], lhsT=ident[:],
                                             rhs=xj_od[:, 0, 0:4],
                                             start=True, stop=True)
                        den = work.tile([P, 4], dt.float32, tag="den")
                        nc.vector.tensor_scalar(
                            out=den[:], in0=msgden[:, HC:HC + 4], scalar1=1e-20,
                            scalar2=None, op0=ALU.max)
                        rden = work.tile([P, 4], dt.float32, tag="rden")
                        nc.vector.reciprocal(rden[:], den[:])
                        orl = work.tile([P, HC], dt.float32, tag="orl")
                        for h in range(H):
                            nc.scalar.activation(
                                orl[:, h * C64:(h + 1) * C64],
                                msgden[:, h * C64:(h + 1) * C64],
                                AF.Relu, scale=rden[:, h:h + 1])
                        for half in range(2):
                            trp = eps.tile([P, P], dt.float32, tag="trp", bufs=1)
                            nc.tensor.transpose(
                                out=trp[:], in_=orl[:, half * P:(half + 1) * P],
                                identity=ident[:])
                            trs = work.tile([P, P], dt.float32, tag="trs")
                            nc.vector.tensor_copy(trs[:], trp[:])
                            nc.sync.dma_start(
                                out=relu_T[half * P:(half + 1) * P,
                                           j * P:(j + 1) * P],
                                in_=trs[:])

            # ---------------- phase A
            with tc.tile_pool(name="pha", bufs=1) as pha:
                xT = pha.tile([P, SHARD], dt.float32, tag="xT")
                nc.sync.dma_start(out=xT[:], in_=x_perm_T[:])
                linear_phase([xT], [wtile["w1"]], bias_t["b1"], h_shard, HC)

            nc.gpsimd.collective_compute(
                "AllGather", mybir.AluOpType.bypass, replica_groups=rg,
                ins=[h_shard.ap().opt()], outs=[h_full.ap().opt()])
            if upto == "A":
                # dump: own shard row 0 block + a remote block from h_full
                nc.sync.dma_start(out=dbg_h[0:P, :], in_=h_shard[0:P, :])
                nc.sync.dma_start(out=dbg_h[P:2 * P, :], in_=h_full[3 * SHARD:3 * SHARD + P, :])
                nc.sync.dma_start(out=y_shard[0:P, :], in_=h_shard[0:P, 0:OUT])
            if upto in ("A",):
                pass
            else:
                edge_layer(att_t[1])

            if upto == "L1":
                nc.sync.dma_start(out=dbg_h[0:P, :], in_=relu_T[0:P, 0:HC])
                nc.sync.dma_start(out=dbg_h[P:2 * P, :], in_=relu_T[P:2 * P, 0:HC])
                nc.sync.dma_start(out=y_shard[0:P, :], in_=relu_T[0:P, 0:OUT])
            if upto != "L1":
                # ---------------- phase B
                with tc.tile_pool(name="phb", bufs=1) as phb:
                    rlo = phb.tile([P, SHARD], dt.float32, tag="rlo")
                    nc.sync.dma_start(out=rlo[:], in_=relu_T[0:P, :])
                    rhi = phb.tile([P, SHARD], dt.float32, tag="rhi")
                    nc.sync.dma_start(out=rhi[:], in_=relu_T[P:2 * P, :])
                    linear_phase([rlo, rhi], [wtile["w2lo"], wtile["w2hi"]],
                                 bias_t["b2"], h_shard, HC)

                if upto == "B":
                    nc.sync.dma_start(out=dbg_h[0:P, :], in_=h_shard[0:P, :])
                    nc.sync.dma_start(out=dbg_h[P:2 * P, :], in_=h_shard[P:2 * P, :])
                    nc.sync.dma_start(out=y_shard[0:P, :], in_=h_shard[0:P, 0:OUT])
            if upto not in ("L1", "B"):
                nc.gpsimd.collective_compute(
                    "AllGather", mybir.AluOpType.bypass, replica_groups=rg,
                    ins=[h_shard.ap().opt()], outs=[h_full.ap().opt()])
                edge_layer(att_t[2])
                if upto == "L2":
                    nc.sync.dma_start(out=dbg_h[0:P, :], in_=relu_T[0:P, 0:HC])
                    nc.sync.dma_start(out=dbg_h[P:2 * P, :], in_=relu_T[P:2 * P, 0:HC])
                    nc.sync.dma_start(out=y_shard[0:P, :], in_=relu_T[0:P, 0:OUT])

            if upto == "full":
                # ---------------- phase C
                with (
                    tc.tile_pool(name="phc", bufs=1) as phc,
                    tc.tile_pool(name="phcs", bufs=3) as phcs,
                    tc.tile_pool(name="phcp", bufs=2, space="PSUM") as phcp,
                ):
                    r2lo = phc.tile([P, SHARD], dt.float32, tag="rlo")
                    nc.sync.dma_start(out=r2lo[:], in_=relu_T[0:P, :])
                    r2hi = phc.tile([P, SHARD], dt.float32, tag="rhi")
                    nc.sync.dma_start(out=r2hi[:], in_=relu_T[P:2 * P, :])
                    for j in range(CHUNKS):
                        ps3 = phcp.tile([P, OUT], dt.float32, tag="ps3")
                        nc.tensor.matmul(out=ps3[:], lhsT=r2lo[:, j * P:(j + 1) * P],
                                         rhs=wtile["w3lo"][:], start=True, stop=False)
                        nc.tensor.matmul(out=ps3[:], lhsT=r2hi[:, j * P:(j + 1) * P],
                                         rhs=wtile["w3hi"][:], start=False, stop=False)
                        nc.tensor.matmul(out=ps3[:], lhsT=ones_row[:],
                                         rhs=bias_t["b3"][:], start=False, stop=True)
                        h3 = phcs.tile([P, OUT], dt.float32, tag="h3")
                        nc.scalar.activation(h3[:], ps3[:], AF.Copy)
                        h3tp = phcp.tile([OUT, P], dt.float32, tag="h3tp")
                        nc.tensor.transpose(out=h3tp[:], in_=h3[:], identity=ident[:])
                        h3t = phcs.tile([OUT, P], dt.float32, tag="h3t")
                        nc.vector.tensor_copy(h3t[:], h3tp[:])
                        ps4 = phcp.tile([P, OUT], dt.float32, tag="ps4")
                        nc.tensor.matmul(out=ps4[:], lhsT=h3t[:], rhs=wtile["w4"][:],
                                         start=True, stop=False)
                        nc.tensor.matmul(out=ps4[:], lhsT=ones_row[:],
                                         rhs=bias_t["b4"][:], start=False, stop=True)
                        yt = phcs.tile([P, OUT], dt.float32, tag="yt")
                        nc.scalar.activation(yt[:], ps4[:], AF.Copy)
                        nc.sync.dma_start(out=y_shard[j * P:(j + 1) * P, :], in_=yt[:])

    nc.compile()
    return nc


# ----------------------------------------------------------------- kernel()

_CACHE = {}


def kernel(**inputs):
    from concourse.bass_utils import run_bass_kernel_spmd

    in_maps, perms, t_ev, t_od = prepare(inputs)
    key = (t_ev, t_od)
    if key not in _CACHE:
        _CACHE[key] = build(t_ev, t_od)
    nc = _CACHE[key]
    res = run_bass_kernel_spmd(nc, in_maps, core_ids=list(range(NCORES)))
    out = np.zeros((N, OUT), np.float32)
    for c in range(NCORES):
        ys = res.results[c]["y_shard"]
        valid = perms[c] >= 0
        out[perms[c][valid] + c * NPC] = ys[valid]
    return out


if __name__ == "__main__":
    import jax
    import reference
    cpu = jax.devices("cpu")[0]
    with jax.default_device(cpu):
        inputs = {k: np.asarray(v) for k, v in reference.setup_inputs().items()}
        exp = np.asarray(reference.reference(**inputs))
    got = kernel(**inputs)
    rel = np.linalg.norm(got - exp) / np.linalg.norm(exp)
    print("Relative error:", rel)



# revision 2
# speedup vs baseline: 2.8480x; 2.8480x over previous
"""GATv2 2-layer GNN on 8 Trainium2 NeuronCores (Bass/Tile), v2.

Strategy (dst-sharded edge parallelism):
- Nodes sharded by destination across 8 cores (6250/core); each core owns all
  edges into its nodes, so segment softmax and aggregation are core-local.
  Nodes are LPT-packed into 49 chunks of 128 slots to balance edge counts.
- Layer 1 needs no device-side gather at all: scores are
  (x[dst]+x[src]) @ W1 + 2 b1 and messages are x[src] @ W1 + b1, so the host
  ships edge-ordered bf16 streams of x[dst]+x[src] and x[src] (feature-major),
  and the PE projects each 128-edge block directly. No projection phase, no
  first AllGather.
- Layer 2: h2 = relu1 @ W2 + b2 is computed per shard (phase B), AllGathered
  (bf16), then per chunk the source rows are fetched with gpsimd dma_gather
  (int16 pair-index trick; even/odd slot parity selects the base offset,
  num_idxs_reg trimmed to the actual edge count). xi comes from a one-hot
  matmul against the chunk's own rows.
- Edge math per 128-edge block: bf16 matmuls into fp32 PSUM, Prelu(0.2) on
  ACT, att-dot via bf16 mul+reduce on DVE, Exp on ACT, bf16 message scaling,
  one-hot scatter matmul accumulating messages + softmax denominators.
  One-hot masks are built on DVE (not gpsimd - the Q7 is reserved for the
  layer-2 gathers, which are its serial bottleneck).
- Epilogue per chunk: divide by denominator, ReLU, write transposed bf16
  copies for the next linear layer. Phase C applies the two post-MP linears.
"""

import numpy as np

N = 50000
E = 800000
IN = 128
HC = 256          # H * HID
H = 4
C64 = 64
OUT = 64
SLOPE = 0.2
NCORES = 8
NPC = N // NCORES          # 6250 nodes per core
CHUNKS = 49                # ceil(6250/128)
P = 128
SHARD = CHUNKS * P         # 6272 slots per core
GSLOTS = NCORES * SHARD    # 50176
PAD_DST = 255.0            # dst slot sentinel for pad edges


# ---------------------------------------------------------------- host prep

def _pack_core(dst_local, n_nodes=NPC, chunks=CHUNKS):
    """LPT-pack nodes into `chunks` bins of <=128 nodes, balancing edges."""
    deg = np.bincount(dst_local, minlength=n_nodes)
    order = np.argsort(-deg, kind="stable")
    bin_load = np.zeros(chunks, np.int64)
    bin_cnt = np.zeros(chunks, np.int32)
    bin_members = [[] for _ in range(chunks)]
    for v in order:
        cand = np.where(bin_cnt < P)[0]
        b = cand[np.argmin(bin_load[cand])]
        bin_members[b].append(v)
        bin_load[b] += deg[v]
        bin_cnt[b] += 1
    perm = np.full(chunks * P, -1, np.int64)
    for b in range(chunks):
        for k, v in enumerate(bin_members[b]):
            perm[b * P + k] = v
    return perm


def _wrap_idx(flat):
    """[n] -> [128, n//16] int16 wrapped (i at [i%16, i//16]) + 8x replicated."""
    n = flat.shape[0]
    w = flat.reshape(n // 16, 16).T.astype(np.int16)
    return np.tile(w, (8, 1)).copy()


def _group_ranks(key, nkeys):
    """Per-element rank within its key group (stable, vectorized)."""
    order = np.argsort(key, kind="stable")
    cnt = np.bincount(key, minlength=nkeys)
    starts = np.zeros(nkeys + 1, np.int64)
    np.cumsum(cnt, out=starts[1:])
    rank = np.empty(key.shape[0], np.int64)
    rank[order] = np.arange(key.shape[0]) - starts[key[order]]
    return rank, cnt


def prepare(inputs):
    import ml_dtypes
    bf16 = ml_dtypes.bfloat16
    x = np.asarray(inputs["x"], np.float32)
    ei = np.asarray(inputs["edge_index"]).astype(np.int64)
    src, dst = ei[0], ei[1]
    owner = dst // NPC
    dst_local = dst - owner * NPC

    perms = []
    pos_of = np.empty(N, np.int64)
    for c in range(NCORES):
        m = owner == c
        perm = _pack_core(dst_local[m])
        perms.append(perm)
        valid = perm >= 0
        pos_of[perm[valid] + c * NPC] = np.nonzero(valid)[0] + c * SHARD
    gsrc = pos_of[src]
    gdst = pos_of[dst]

    chunk = (gdst % SHARD) // P
    dslot = gdst % P

    # ---- layer 1: edge-ordered streams, no parity split -------------------
    key1 = (owner * CHUNKS + chunk).astype(np.int64)
    rank1, cnt1 = _group_ranks(key1, NCORES * CHUNKS)
    cnt1_cj = cnt1.reshape(NCORES, CHUNKS)
    t1 = np.ceil(cnt1_cj.max(axis=0) / P).astype(np.int64)   # blocks per chunk
    t1 = np.maximum(t1, 1)
    off1 = np.zeros(CHUNKS + 1, np.int64)
    np.cumsum(t1, out=off1[1:])
    TOTB1 = int(off1[-1])
    NS1 = TOTB1 * P

    slot1 = off1[chunk] * P + rank1
    dmt1 = np.full((NCORES, 128, TOTB1), PAD_DST, np.float32)
    dmt1[owner, rank1 % P, off1[chunk] + rank1 // P] = dslot

    xsum_s = np.zeros((NCORES, IN, NS1), bf16)
    xsrc_s = np.zeros((NCORES, IN, NS1), bf16)
    for c in range(NCORES):
        m = owner == c
        sl = slot1[m]
        xs = x[src[m]]
        xd = x[dst[m]]
        a = np.zeros((NS1, IN), np.float32)
        a[sl] = xs + xd
        xsum_s[c] = a.T.astype(bf16)
        a[:] = 0
        a[sl] = xs
        xsrc_s[c] = a.T.astype(bf16)

    # ---- layer 2: gather indices, parity split ----------------------------
    par = (gsrc & 1).astype(np.int64)
    key2 = ((owner * CHUNKS + chunk) * 2 + par).astype(np.int64)
    rank2, cnt2 = _group_ranks(key2, NCORES * CHUNKS * 2)
    cnt2_cjp = cnt2.reshape(NCORES, CHUNKS, 2)
    t_ev = np.maximum(np.ceil(cnt2_cjp[:, :, 0].max(axis=0) / P), 1).astype(np.int64)
    t_od = np.maximum(np.ceil(cnt2_cjp[:, :, 1].max(axis=0) / P), 1).astype(np.int64)
    reg_ev = cnt2_cjp[:, :, 0].max(axis=0).astype(np.int64)   # per chunk
    reg_od = cnt2_cjp[:, :, 1].max(axis=0).astype(np.int64)
    off_ev = np.zeros(CHUNKS + 1, np.int64)
    np.cumsum(t_ev, out=off_ev[1:])
    off_od = np.zeros(CHUNKS + 1, np.int64)
    np.cumsum(t_od, out=off_od[1:])
    TOT_EV = int(off_ev[-1])
    TOT_OD = int(off_od[-1])
    t2 = t_ev + t_od
    off2 = np.zeros(CHUNKS + 1, np.int64)
    np.cumsum(t2, out=off2[1:])
    TOTB2 = int(off2[-1])

    ev_idx = np.zeros((NCORES, 128, TOT_EV * 8), np.int16)
    od_idx = np.zeros((NCORES, 128, TOT_OD * 8), np.int16)
    dmt2 = np.full((NCORES, 128, TOTB2), PAD_DST, np.float32)

    colbase = np.where(par == 0, off2[chunk], off2[chunk] + t_ev[chunk])
    dmt2[owner, rank2 % P, colbase + rank2 // P] = dslot

    for c in range(NCORES):
        for p, (arr, offs, ts) in enumerate(((ev_idx, off_ev, t_ev),
                                             (od_idx, off_od, t_od))):
            m = (owner == c) & (par == p)
            ch = chunk[m]
            rk = rank2[m]
            gs = gsrc[m] >> 1
            for j in range(CHUNKS):
                mj = ch == j
                flat = np.zeros(int(ts[j]) * P, np.int64)
                flat[rk[mj]] = gs[mj]
                arr[c, :, offs[j] * 8:(offs[j] + ts[j]) * 8] = _wrap_idx(flat)

    # ---- weights ----------------------------------------------------------
    W1 = np.asarray(inputs["W1"], np.float32)
    W2 = np.asarray(inputs["W2"], np.float32)
    W3 = np.asarray(inputs["W3"], np.float32)
    W4 = np.asarray(inputs["W4"], np.float32)
    b1 = np.asarray(inputs["b1"], np.float32)
    b2 = np.asarray(inputs["b2"], np.float32)
    b3 = np.asarray(inputs["b3"], np.float32)
    b4 = np.asarray(inputs["b4"], np.float32)
    has_b = (bool(b1.any()), bool(b2.any()), bool(b3.any()), bool(b4.any()))

    att1b = np.tile(np.asarray(inputs["att1"], np.float32).reshape(1, 1, HC),
                    (P, 1, 1)).astype(bf16)
    att2b = np.tile(np.asarray(inputs["att2"], np.float32).reshape(1, 1, HC),
                    (P, 1, 1)).astype(bf16)

    common = {
        "W1b": np.ascontiguousarray(W1.T).astype(bf16),
        "W2b": np.ascontiguousarray(W2.T).astype(bf16),
        "W3b": np.ascontiguousarray(W3.T).astype(bf16),
        "W4b": np.ascontiguousarray(W4.T).astype(bf16),
        "b1x2": (2 * b1).reshape(1, HC).astype(bf16),
        "b1r": b1.reshape(1, HC).astype(bf16),
        "b2r": b2.reshape(1, HC).astype(bf16),
        "b3r": b3.reshape(1, OUT).astype(bf16),
        "b4r": b4.reshape(1, OUT).astype(bf16),
        "att1_b": att1b, "att2_b": att2b,
    }
    in_maps = []
    for c in range(NCORES):
        d = dict(common)
        d.update({
            "xsum_s": xsum_s[c], "xsrc_s": xsrc_s[c],
            "dmt1_d": dmt1[c], "dmt2_d": dmt2[c],
            "evi_d": ev_idx[c], "odi_d": od_idx[c],
        })
        in_maps.append(d)

    meta = {
        "t1": tuple(int(v) for v in t1),
        "off1": tuple(int(v) for v in off1),
        "t_ev": tuple(int(v) for v in t_ev),
        "t_od": tuple(int(v) for v in t_od),
        "reg_ev": tuple(int(v) for v in reg_ev),
        "reg_od": tuple(int(v) for v in reg_od),
        "off_ev": tuple(int(v) for v in off_ev),
        "off_od": tuple(int(v) for v in off_od),
        "off2": tuple(int(v) for v in off2),
        "has_b": has_b,
    }
    return in_maps, perms, meta


# ------------------------------------------------------------- device build

def build(meta):
    import concourse.bacc as bacc
    import concourse.mybir as mybir
    import concourse.tile as tile
    from concourse.masks import make_identity

    dt = mybir.dt
    AF = mybir.ActivationFunctionType
    ALU = mybir.AluOpType
    AX = mybir.AxisListType

    t1 = meta["t1"]
    off1 = meta["off1"]
    t_ev, t_od = meta["t_ev"], meta["t_od"]
    reg_ev, reg_od = meta["reg_ev"], meta["reg_od"]
    off_ev, off_od, off2 = meta["off_ev"], meta["off_od"], meta["off2"]
    has_b1, has_b2, has_b3, has_b4 = meta["has_b"]
    TOTB1 = off1[-1]
    TOT_EV, TOT_OD, TOTB2 = off_ev[-1], off_od[-1], off2[-1]
    T1MAX = max(t1)
    TEVMAX, TODMAX = max(t_ev), max(t_od)

    nc = bacc.Bacc("TRN2", target_bir_lowering=False, debug=False,
                   num_devices=NCORES)

    xsum_s = nc.dram_tensor("xsum_s", [IN, TOTB1 * P], dt.bfloat16, kind="ExternalInput")
    xsrc_s = nc.dram_tensor("xsrc_s", [IN, TOTB1 * P], dt.bfloat16, kind="ExternalInput")
    dmt1_d = nc.dram_tensor("dmt1_d", [128, TOTB1], dt.float32, kind="ExternalInput")
    dmt2_d = nc.dram_tensor("dmt2_d", [128, TOTB2], dt.float32, kind="ExternalInput")
    evi_d = nc.dram_tensor("evi_d", [128, TOT_EV * 8], dt.int16, kind="ExternalInput")
    odi_d = nc.dram_tensor("odi_d", [128, TOT_OD * 8], dt.int16, kind="ExternalInput")
    W1b = nc.dram_tensor("W1b", [IN, HC], dt.bfloat16, kind="ExternalInput")
    W2b = nc.dram_tensor("W2b", [HC, HC], dt.bfloat16, kind="ExternalInput")
    W3b = nc.dram_tensor("W3b", [HC, OUT], dt.bfloat16, kind="ExternalInput")
    W4b = nc.dram_tensor("W4b", [OUT, OUT], dt.bfloat16, kind="ExternalInput")
    b1x2 = nc.dram_tensor("b1x2", [1, HC], dt.bfloat16, kind="ExternalInput")
    b1r = nc.dram_tensor("b1r", [1, HC], dt.bfloat16, kind="ExternalInput")
    b2r = nc.dram_tensor("b2r", [1, HC], dt.bfloat16, kind="ExternalInput")
    b3r = nc.dram_tensor("b3r", [1, OUT], dt.bfloat16, kind="ExternalInput")
    b4r = nc.dram_tensor("b4r", [1, OUT], dt.bfloat16, kind="ExternalInput")
    att1_b = nc.dram_tensor("att1_b", [P, 1, HC], dt.bfloat16, kind="ExternalInput")
    att2_b = nc.dram_tensor("att2_b", [P, 1, HC], dt.bfloat16, kind="ExternalInput")
    y_shard = nc.dram_tensor("y_shard", [SHARD, OUT], dt.float32, kind="ExternalOutput")

    h_shard = nc.dram_tensor("h_shard", [SHARD, HC], dt.bfloat16)
    h_full = nc.dram_tensor("h_full", [GSLOTS, HC], dt.bfloat16, addr_space="Shared")
    relu_T = nc.dram_tensor("relu_T", [HC, SHARD], dt.bfloat16)

    rg = [list(range(NCORES))]

    with tile.TileContext(nc, num_cores=NCORES) as tc:
        with tc.tile_pool(name="const", bufs=1) as constp:
            ident = constp.tile([P, P], dt.float32)
            make_identity(nc, ident[:])
            identb = constp.tile([P, P], dt.bfloat16)
            nc.vector.tensor_copy(identb[:], ident[:])
            iota_col = constp.tile([P, 1], dt.float32)
            nc.gpsimd.iota(iota_col[:], pattern=[[0, 1]], base=0,
                           channel_multiplier=1,
                           allow_small_or_imprecise_dtypes=True)
            iota_row = constp.tile([P, P], dt.float32)
            nc.gpsimd.iota(iota_row[:], pattern=[[1, P]], base=0,
                           channel_multiplier=0,
                           allow_small_or_imprecise_dtypes=True)
            iota_row_w = constp.tile([P, 3, P], dt.float32)
            nc.vector.tensor_copy(
                iota_row_w[:],
                iota_row[:].rearrange("p (o q) -> p o q", o=1)
                .to_broadcast([P, 3, P]))
            ones_row = constp.tile([1, P], dt.bfloat16)
            nc.gpsimd.memset(ones_row[:], 1.0)

            att_t = {1: constp.tile([P, 1, HC], dt.bfloat16, name="att_t1"),
                     2: constp.tile([P, 1, HC], dt.bfloat16, name="att_t2")}
            nc.sync.dma_start(out=att_t[1][:], in_=att1_b[:])
            nc.sync.dma_start(out=att_t[2][:], in_=att2_b[:])
            bias_t = {}
            for name, t_, w in (("b1x2", b1x2, HC), ("b1", b1r, HC),
                                ("b2", b2r, HC), ("b3", b3r, OUT),
                                ("b4", b4r, OUT)):
                bt = constp.tile([1, w], dt.bfloat16, name=f"bias_{name}")
                nc.sync.dma_start(out=bt[:], in_=t_[:])
                bias_t[name] = bt
            wtile = {}
            for name, t_, kk, w in (("w1", W1b, IN, HC),
                                    ("w2lo", W2b[0:P, :], P, HC),
                                    ("w2hi", W2b[P:2 * P, :], P, HC),
                                    ("w3lo", W3b[0:P, :], P, OUT),
                                    ("w3hi", W3b[P:2 * P, :], P, OUT),
                                    ("w4", W4b, OUT, OUT)):
                wt = constp.tile([kk, w], dt.bfloat16, name=f"w_{name}")
                nc.sync.dma_start(out=wt[:], in_=t_ if name not in ("w1", "w4") else t_[:])
                wtile[name] = wt
            dmt1 = constp.tile([128, TOTB1], dt.float32, name="dmt1")
            nc.sync.dma_start(out=dmt1[:], in_=dmt1_d[:])
            dmt2 = constp.tile([128, TOTB2], dt.float32, name="dmt2")
            nc.sync.dma_start(out=dmt2[:], in_=dmt2_d[:])

            def edge_epilogue(work, eps, msgden, j):
                den = work.tile([P, 4], dt.float32, tag="den")
                nc.vector.tensor_scalar(
                    out=den[:], in0=msgden[:, HC:HC + 4], scalar1=1e-20,
                    scalar2=None, op0=ALU.max)
                rden = work.tile([P, 4], dt.float32, tag="rden")
                nc.vector.reciprocal(rden[:], den[:])
                orl = work.tile([P, HC], dt.float32, tag="orl")
                for h in range(H):
                    nc.scalar.activation(
                        orl[:, h * C64:(h + 1) * C64],
                        msgden[:, h * C64:(h + 1) * C64],
                        AF.Relu, scale=rden[:, h:h + 1])
                for half in range(2):
                    trp = eps.tile([P, P], dt.float32, tag="trp", bufs=1)
                    nc.tensor.transpose(
                        out=trp[:], in_=orl[:, half * P:(half + 1) * P],
                        identity=ident[:])
                    trs = work.tile([P, P], dt.bfloat16, tag="trs")
                    nc.vector.tensor_copy(trs[:], trp[:])
                    nc.sync.dma_start(
                        out=relu_T[half * P:(half + 1) * P,
                                   j * P:(j + 1) * P],
                        in_=trs[:])

            # ================= layer 1: host-streamed edge phase ===========
            with (
                tc.tile_pool(name="l1io", bufs=2) as l1io,
                tc.tile_pool(name="l1w", bufs=2) as work,
                tc.tile_pool(name="l1ps", bufs=1, space="PSUM") as eps,
            ):
                for j in range(CHUNKS):
                    nb = t1[j]
                    xsumt = l1io.tile([IN, T1MAX * P], dt.bfloat16, tag="xsum")
                    nc.sync.dma_start(out=xsumt[:, 0:nb * P],
                                      in_=xsum_s[:, off1[j] * P:(off1[j] + nb) * P])
                    xsrct = l1io.tile([IN, T1MAX * P], dt.bfloat16, tag="xsrc")
                    nc.sync.dma_start(out=xsrct[:, 0:nb * P],
                                      in_=xsrc_s[:, off1[j] * P:(off1[j] + nb) * P])

                    msgden = eps.tile([P, HC + 4], dt.float32, tag="msgden",
                                      bufs=1)
                    first = True
                    t0 = 0
                    while t0 < nb:
                        gb = min(3, nb - t0)
                        zpk = eps.tile([P, 6 * HC], dt.float32, tag="zpk",
                                       bufs=2)
                        zc = zpk[:, 0:3 * HC]
                        mp = zpk[:, 3 * HC:6 * HC]
                        for i in range(gb):
                            nc.tensor.matmul(
                                out=zc[:, i * HC:(i + 1) * HC],
                                lhsT=xsumt[:, (t0 + i) * P:(t0 + i + 1) * P],
                                rhs=wtile["w1"][:], start=True, stop=not has_b1)
                            if has_b1:
                                nc.tensor.matmul(
                                    out=zc[:, i * HC:(i + 1) * HC],
                                    lhsT=ones_row[:], rhs=bias_t["b1x2"][:],
                                    start=False, stop=True)
                            nc.tensor.matmul(
                                out=mp[:, i * HC:(i + 1) * HC],
                                lhsT=xsrct[:, (t0 + i) * P:(t0 + i + 1) * P],
                                rhs=wtile["w1"][:], start=True, stop=not has_b1)
                            if has_b1:
                                nc.tensor.matmul(
                                    out=mp[:, i * HC:(i + 1) * HC],
                                    lhsT=ones_row[:], rhs=bias_t["b1"][:],
                                    start=False, stop=True)
                        s_b = work.tile([P, 3 * HC], dt.bfloat16, tag="s_b")
                        nc.scalar.activation(s_b[:, 0:gb * HC], zc[:, 0:gb * HC],
                                             AF.Prelu, alpha=SLOPE)
                        t_b = work.tile([P, 3 * HC], dt.bfloat16, tag="t_b")
                        nc.vector.tensor_tensor(
                            out=t_b[:, 0:gb * HC].rearrange("p (g d) -> p g d", g=gb),
                            in0=s_b[:, 0:gb * HC].rearrange("p (g d) -> p g d", g=gb),
                            in1=att_t[1][:].to_broadcast([P, gb, HC]),
                            op=ALU.mult)
                        alph = work.tile([P, 12], dt.float32, tag="alph")
                        nc.vector.tensor_reduce(
                            out=alph[:, 0:gb * 4].rearrange("p (g h) -> p g h", g=gb),
                            in_=t_b[:, 0:gb * HC].rearrange("p (g h c) -> p g h c",
                                                            g=gb, h=H),
                            axis=AX.X, op=ALU.add)
                        msge = work.tile([P, 3, HC + 4], dt.bfloat16, tag="msge")
                        ef = work.tile([P, 12], dt.float32, tag="ef")
                        nc.scalar.activation(
                            ef[:, 0:gb * 4], alph[:, 0:gb * 4], AF.Exp)
                        nc.vector.tensor_copy(
                            msge[:, 0:gb, HC:HC + 4],
                            ef[:, 0:gb * 4].rearrange("p (g h) -> p g h", g=gb))
                        nc.vector.tensor_tensor(
                            out=msge[:, 0:gb, 0:HC].rearrange(
                                "p g (h c) -> p g h c", h=H),
                            in0=mp[:, 0:gb * HC].rearrange(
                                "p (g h c) -> p g h c", g=gb, h=H),
                            in1=ef[:, 0:gb * 4].rearrange("p (g h) -> p g h", g=gb)
                            .to_broadcast([P, gb, H, C64]),
                            op=ALU.mult)
                        s_en = work.tile([P, 3 * P], dt.bfloat16, tag="s_en")
                        nc.vector.tensor_tensor(
                            out=s_en[:, 0:gb * P].rearrange("p (g q) -> p g q", g=gb),
                            in0=iota_row_w[:, 0:gb, :],
                            in1=dmt1[:, off1[j] + t0:off1[j] + t0 + gb]
                            .rearrange("p (g o) -> p g o", o=1)
                            .to_broadcast([P, gb, P]),
                            op=ALU.is_equal)
                        for i in range(gb):
                            nc.tensor.matmul(
                                out=msgden[:], lhsT=s_en[:, i * P:(i + 1) * P],
                                rhs=msge[:, i, :],
                                start=first, stop=(t0 + i == nb - 1))
                            first = False
                        t0 += gb
                    edge_epilogue(work, eps, msgden, j)

            # ================= phase B: h2 = relu1 @ W2 + b2 ===============
            with (
                tc.tile_pool(name="phb", bufs=1) as phb,
                tc.tile_pool(name="phbs", bufs=3) as phbs,
                tc.tile_pool(name="phbp", bufs=2, space="PSUM") as phbp,
            ):
                r1lo = phb.tile([P, SHARD], dt.bfloat16, tag="rlo")
                nc.sync.dma_start(out=r1lo[:], in_=relu_T[0:P, :])
                r1hi = phb.tile([P, SHARD], dt.bfloat16, tag="rhi")
                nc.sync.dma_start(out=r1hi[:], in_=relu_T[P:2 * P, :])
                for j in range(CHUNKS):
                    ps = phbp.tile([P, HC], dt.float32, tag="psb")
                    nc.tensor.matmul(out=ps[:], lhsT=r1lo[:, j * P:(j + 1) * P],
                                     rhs=wtile["w2lo"][:], start=True, stop=False)
                    nc.tensor.matmul(out=ps[:], lhsT=r1hi[:, j * P:(j + 1) * P],
                                     rhs=wtile["w2hi"][:], start=False,
                                     stop=not has_b2)
                    if has_b2:
                        nc.tensor.matmul(out=ps[:], lhsT=ones_row[:],
                                         rhs=bias_t["b2"][:], start=False,
                                         stop=True)
                    hsb = phbs.tile([P, HC], dt.bfloat16, tag="hsb")
                    nc.scalar.activation(hsb[:], ps[:], AF.Copy)
                    nc.sync.dma_start(out=h_shard[j * P:(j + 1) * P, :],
                                      in_=hsb[:])

            nc.gpsimd.collective_compute(
                "AllGather", mybir.AluOpType.bypass, replica_groups=rg,
                ins=[h_shard.ap().opt()], outs=[h_full.ap().opt()])

            # ================= layer 2: gather edge phase ==================
            pairs = h_full[:].rearrange("(a b) d -> a (b d)", b=2)
            with (
                tc.tile_pool(name="l2io", bufs=2) as l2io,
                tc.tile_pool(name="l2w", bufs=2) as work,
                tc.tile_pool(name="l2ps", bufs=1, space="PSUM") as eps,
            ):
                # pre-touch both rotation buffers of the gather tiles so
                # num_idxs_reg-trimmed tails read finite stale data
                for _ in range(2):
                    ze = l2io.tile([P, TEVMAX, HC], dt.bfloat16, tag="xjev")
                    nc.vector.memset(ze[:, 0:1, :], 0.0)
                    zo = l2io.tile([P, TODMAX, HC], dt.bfloat16, tag="xjod")
                    nc.vector.memset(zo[:, 0:1, :], 0.0)

                for j in range(CHUNKS):
                    tev, tod = t_ev[j], t_od[j]
                    nb = tev + tod
                    hck = l2io.tile([P, HC], dt.bfloat16, tag="hchunk")
                    nc.sync.dma_start(out=hck[:],
                                      in_=h_shard[j * P:(j + 1) * P, :])
                    evi = l2io.tile([128, TEVMAX * 8], dt.int16, tag="evi")
                    nc.sync.dma_start(out=evi[:, 0:tev * 8],
                                      in_=evi_d[:, off_ev[j] * 8:(off_ev[j] + tev) * 8])
                    odi = l2io.tile([128, TODMAX * 8], dt.int16, tag="odi")
                    nc.sync.dma_start(out=odi[:, 0:tod * 8],
                                      in_=odi_d[:, off_od[j] * 8:(off_od[j] + tod) * 8])
                    xj_ev = l2io.tile([P, TEVMAX, HC], dt.bfloat16, tag="xjev")
                    xj_od = l2io.tile([P, TODMAX, HC], dt.bfloat16, tag="xjod")
                    nc.gpsimd.dma_gather(
                        out_ap=xj_ev[:, 0:tev, :], in_ap=pairs[:, 0:HC],
                        idxs_ap=evi[:, 0:tev * 8],
                        num_idxs=tev * P, num_idxs_reg=max(reg_ev[j], 1),
                        elem_size=HC, elem_step=2 * HC, single_packet=False)
                    nc.gpsimd.dma_gather(
                        out_ap=xj_od[:, 0:tod, :], in_ap=pairs[:, HC:2 * HC],
                        idxs_ap=odi[:, 0:tod * 8],
                        num_idxs=tod * P, num_idxs_reg=max(reg_od[j], 1),
                        elem_size=HC, elem_step=2 * HC, single_packet=False)

                    msgden = eps.tile([P, HC + 4], dt.float32, tag="msgden",
                                      bufs=1)
                    groups = []
                    for base, tcnt, pool in ((0, tev, xj_ev),
                                             (tev, tod, xj_od)):
                        t0 = 0
                        while t0 < tcnt:
                            gb = min(3, tcnt - t0)
                            groups.append((base, t0, gb, pool))
                            t0 += gb
                    first = True
                    for (base, t0, gb, pool) in groups:
                        c0 = off2[j] + base + t0
                        dstb = eps.tile([P, 3 * P], dt.float32, tag="dstb",
                                        bufs=1)
                        for i in range(gb):
                            nc.tensor.transpose(
                                out=dstb[:, i * P:(i + 1) * P],
                                in_=dmt2[:, c0 + i:c0 + i + 1]
                                    .to_broadcast([P, P]),
                                identity=ident[:])
                        sTb = work.tile([P, 3 * P], dt.bfloat16, tag="sTb")
                        nc.vector.tensor_scalar(
                            out=sTb[:, 0:gb * P], in0=dstb[:, 0:gb * P],
                            scalar1=iota_col[:, :1],
                            scalar2=None, op0=ALU.is_equal)
                        zp = eps.tile([P, 3 * HC], dt.float32, tag="zp",
                                      bufs=2)
                        for i in range(gb):
                            nc.tensor.matmul(
                                out=zp[:, i * HC:(i + 1) * HC],
                                lhsT=sTb[:, i * P:(i + 1) * P], rhs=hck[:],
                                start=True, stop=False)
                            nc.tensor.matmul(
                                out=zp[:, i * HC:(i + 1) * HC],
                                lhsT=identb[:], rhs=pool[:, t0 + i, :],
                                start=False, stop=True)
                        s_b = work.tile([P, 3 * HC], dt.bfloat16, tag="s_b")
                        nc.scalar.activation(s_b[:, 0:gb * HC], zp[:, 0:gb * HC],
                                             AF.Prelu, alpha=SLOPE)
                        t_b = work.tile([P, 3 * HC], dt.bfloat16, tag="t_b")
                        nc.vector.tensor_tensor(
                            out=t_b[:, 0:gb * HC].rearrange("p (g d) -> p g d", g=gb),
                            in0=s_b[:, 0:gb * HC].rearrange("p (g d) -> p g d", g=gb),
                            in1=att_t[2][:].to_broadcast([P, gb, HC]),
                            op=ALU.mult)
                        alph = work.tile([P, 12], dt.float32, tag="alph")
                        nc.vector.tensor_reduce(
                            out=alph[:, 0:gb * 4].rearrange("p (g h) -> p g h", g=gb),
                            in_=t_b[:, 0:gb * HC].rearrange("p (g h c) -> p g h c",
                                                            g=gb, h=H),
                            axis=AX.X, op=ALU.add)
                        msge = work.tile([P, 3, HC + 4], dt.bfloat16, tag="msge")
                        nc.scalar.activation(
                            msge[:, 0:gb, HC:HC + 4],
                            alph[:, 0:gb * 4].rearrange("p (g h) -> p g h", g=gb),
                            AF.Exp)
                        nc.vector.tensor_tensor(
                            out=msge[:, 0:gb, 0:HC].rearrange(
                                "p g (h c) -> p g h c", h=H),
                            in0=pool[:, t0:t0 + gb, :].rearrange(
                                "p g (h c) -> p g h c", h=H),
                            in1=msge[:, 0:gb, HC:HC + 4].to_broadcast(
                                [P, gb, H, C64]),
                            op=ALU.mult)
                        s_en = work.tile([P, 3 * P], dt.bfloat16, tag="s_en")
                        nc.vector.tensor_tensor(
                            out=s_en[:, 0:gb * P].rearrange("p (g q) -> p g q", g=gb),
                            in0=iota_row_w[:, 0:gb, :],
                            in1=dmt2[:, c0:c0 + gb]
                            .rearrange("p (g o) -> p g o", o=1)
                            .to_broadcast([P, gb, P]),
                            op=ALU.is_equal)
                        for i in range(gb):
                            nc.tensor.matmul(
                                out=msgden[:], lhsT=s_en[:, i * P:(i + 1) * P],
                                rhs=msge[:, i, :],
                                start=first, stop=(base + t0 + i == nb - 1))
                            first = False
                    edge_epilogue(work, eps, msgden, j)

            # ================= phase C: post-MP linears ====================
            with (
                tc.tile_pool(name="phc", bufs=1) as phc,
                tc.tile_pool(name="phcs", bufs=3) as phcs,
                tc.tile_pool(name="phcp", bufs=2, space="PSUM") as phcp,
            ):
                r2lo = phc.tile([P, SHARD], dt.bfloat16, tag="rlo")
                nc.sync.dma_start(out=r2lo[:], in_=relu_T[0:P, :])
                r2hi = phc.tile([P, SHARD], dt.bfloat16, tag="rhi")
                nc.sync.dma_start(out=r2hi[:], in_=relu_T[P:2 * P, :])
                for j in range(CHUNKS):
                    ps3 = phcp.tile([P, OUT], dt.float32, tag="ps3")
                    nc.tensor.matmul(out=ps3[:], lhsT=r2lo[:, j * P:(j + 1) * P],
                                     rhs=wtile["w3lo"][:], start=True, stop=False)
                    nc.tensor.matmul(out=ps3[:], lhsT=r2hi[:, j * P:(j + 1) * P],
                                     rhs=wtile["w3hi"][:], start=False,
                                     stop=not has_b3)
                    if has_b3:
                        nc.tensor.matmul(out=ps3[:], lhsT=ones_row[:],
                                         rhs=bias_t["b3"][:], start=False,
                                         stop=True)
                    h3 = phcs.tile([P, OUT], dt.bfloat16, tag="h3")
                    nc.scalar.activation(h3[:], ps3[:], AF.Copy)
                    h3tp = phcp.tile([OUT, P], dt.bfloat16, tag="h3tp")
                    nc.tensor.transpose(out=h3tp[:], in_=h3[:], identity=identb[:])
                    h3t = phcs.tile([OUT, P], dt.bfloat16, tag="h3t")
                    nc.vector.tensor_copy(h3t[:], h3tp[:])
                    ps4 = phcp.tile([P, OUT], dt.float32, tag="ps4")
                    nc.tensor.matmul(out=ps4[:], lhsT=h3t[:], rhs=wtile["w4"][:],
                                     start=True, stop=not has_b4)
                    if has_b4:
                        nc.tensor.matmul(out=ps4[:], lhsT=ones_row[:],
                                         rhs=bias_t["b4"][:], start=False,
                                         stop=True)
                    yt = phcs.tile([P, OUT], dt.float32, tag="yt")
                    nc.scalar.activation(yt[:], ps4[:], AF.Copy)
                    nc.sync.dma_start(out=y_shard[j * P:(j + 1) * P, :], in_=yt[:])

    nc.compile()
    return nc


# ----------------------------------------------------------------- kernel()

_CACHE = {}


def kernel(**inputs):
    from concourse.bass_utils import run_bass_kernel_spmd

    in_maps, perms, meta = prepare(inputs)
    key = tuple(sorted(meta.items()))
    if key not in _CACHE:
        _CACHE[key] = build(meta)
    nc = _CACHE[key]
    res = run_bass_kernel_spmd(nc, in_maps, core_ids=list(range(NCORES)))
    out = np.zeros((N, OUT), np.float32)
    for c in range(NCORES):
        ys = res.results[c]["y_shard"]
        valid = perms[c] >= 0
        out[perms[c][valid] + c * NPC] = ys[valid]
    return out


if __name__ == "__main__":
    import jax
    import reference
    cpu = jax.devices("cpu")[0]
    with jax.default_device(cpu):
        inputs = {k: np.asarray(v) for k, v in reference.setup_inputs().items()}
        exp = np.asarray(reference.reference(**inputs))
    got = kernel(**inputs)
    rel = np.linalg.norm(got - exp) / np.linalg.norm(exp)
    print("Relative error:", rel)
